# revision 9
# baseline (speedup 1.0000x reference)
"""AttentionPool Trainium2 Bass kernel (v2: software-pipelined schedule).

Computes, for h:[N,512] f32, sorted batch_vec:[N] int, gate-MLP weights
W1/b1/W2/b2:
    gate  = gelu(h @ W1 + b1) @ W2 + b2            (erf gelu)
    alpha = segment_softmax(gate, batch_vec)       (1024 segments)
    out   = segment_sum(alpha[:,None] * h)         -> [1024, 512]

Sharding: data-parallel over graphs. Core c owns graphs [128c, 128c+128)
and the contiguous node range covering them (batch_vec sorted => segments
never straddle cores).

v2 design (from measured ablations of v1):
  * v1 ran phase A (gate MLP) and phase C (pooling) back-to-back on the
    in-order PE queue with pg (a1@W2) stalling on ACT gelu every
    supertile: 164 us vs ~78 us of actual PE work.  v2 interleaves, per
    supertile slot s: mm1(s) matmuls, pg(s-1) (gelu of s-1 finished
    during mm1(s)), and 4 pooling tiles of supertile s-13 (whose exp was
    produced by a mid-kernel batched Exp).  PE never waits on ACT/DVE.
  * hT is stored in fp8 e3m4 (4 mantissa bits): halves the hT DMA
    stream (measured DMA floor 82us for 26MB; now ~20MB).  Gate-path
    e3m4 error measured 1.07e-2 rel (tolerance 2e-2); pooling h stays
    f16.  Mixed e3m4 x f16 matmuls run at ~141ns per [128,512] tile.
  * gate/e vectors are split into lo/hi tiles at the exp batch boundary
    so pooling of early tiles never waits on the late exp.
  * all DVE ms-build operands are 16-bit (2x DVE rate).
The softmax max-subtraction is skipped: gates are O(1) so exp is safe in
fp32, and the result is mathematically identical.
"""

import os
from contextlib import ExitStack, nullcontext

import numpy as np

import concourse.bass as bass
import concourse.mybir as mybir
from concourse import bacc
import concourse.tile as tile
from concourse.bass_utils import run_bass_kernel_spmd

F32 = mybir.dt.float32
F16 = mybir.dt.float16
F8E3 = mybir.dt.float8e3

N_NODES = 100000
H = 512
NUM_GRAPHS = 1024
N_CORES = 8
G = NUM_GRAPHS // N_CORES  # graphs per core = 128
NP_DEFAULT = 12800         # padded nodes per core (25 supertiles of 512)
KC = H // 128              # contraction chunks = 4

HP_BUFS = int(os.environ.get("AP_HP_BUFS", "15"))
HT_BUFS = int(os.environ.get("AP_HT_BUFS", "4"))
POOL_LAG = int(os.environ.get("AP_POOL_LAG", "1"))  # extra pool slack slots

try:
    import ml_dtypes
    NP_E3 = ml_dtypes.float8_e3m4
except ImportError:  # pragma: no cover
    NP_E3 = None


def _build(np_pad: int, reps: int = 1, ablate: str = "", no_bias: bool = True):
    """Build the per-core Bass program (SPMD: same program, per-core data)."""
    T = np_pad // 128          # 128-node tiles
    S = np_pad // 512          # 512-node supertiles

    # exp batch 1 is issued in slot SPLIT (right after the pg block of
    # supertile SPLIT-1, so it covers gates of supertiles 0..SPLIT-1 =
    # tiles 0..T1-1); pooling of tile t < T1 then runs 1/slot in slots
    # SPLIT+1..S-1; the rest pools in the tail after exp batch 2.
    SPLIT = S // 2
    T1 = 4 * SPLIT

    nc = bacc.Bacc("TRN2", target_bir_lowering=False, debug=False)

    ht_d = nc.dram_tensor("htq", [S, 128, KC * 512], F8E3,
                          kind="ExternalInput")
    hp_d = nc.dram_tensor("hp", [np_pad, H], F16, kind="ExternalInput")
    w1_d = nc.dram_tensor("W1v", [H, H], F16, kind="ExternalInput")
    b1_d = nc.dram_tensor("b1v", [128, KC], F32, kind="ExternalInput")
    w2_d = nc.dram_tensor("W2v", [128, KC * 2], F16, kind="ExternalInput")
    b2_d = nc.dram_tensor("b2t", [128, 1], F32, kind="ExternalInput")
    bv_d = nc.dram_tensor("bvrel", [128, T], F32, kind="ExternalInput")
    io_d = nc.dram_tensor("iota", [128, 128], F16, kind="ExternalInput")
    out_d = nc.dram_tensor("out", [G, H], F32, kind="ExternalOutput")

    gelu_func = mybir.ActivationFunctionType.Gelu
    exp_func = mybir.ActivationFunctionType.Exp

    # python-side pool schedule: slot -> list of supertile indices to pool
    pool_sched = {s: [] for s in range(S)}
    pu_next = 0
    for s in range(SPLIT + 1 + POOL_LAG, S):
        take = []
        while (pu_next < S and 4 * (pu_next + 1) <= T1
               and len(take) < 1 and pu_next <= s - 2):
            take.append(pu_next)
            pu_next += 1
        pool_sched[s] = take
    tail_pus = list(range(pu_next, S))

    with tile.TileContext(nc) as tc, ExitStack() as ctx:
        consts = ctx.enter_context(tc.tile_pool(name="consts", bufs=1))
        ht_pool = ctx.enter_context(tc.tile_pool(name="ht", bufs=HT_BUFS))
        a1_pool = ctx.enter_context(tc.tile_pool(name="a1", bufs=2))
        hp_pool = ctx.enter_context(tc.tile_pool(name="hp", bufs=HP_BUFS))
        ms_pool = ctx.enter_context(tc.tile_pool(name="ms", bufs=6))
        small = ctx.enter_context(tc.tile_pool(name="small", bufs=2))
        psz = ctx.enter_context(tc.tile_pool(
            name="psz", bufs=(2 if no_bias else 4), space="PSUM"))
        psg = ctx.enter_context(tc.tile_pool(name="psg", bufs=2, space="PSUM"))
        psp = ctx.enter_context(tc.tile_pool(name="psp", bufs=1, space="PSUM"))
        psd = ctx.enter_context(tc.tile_pool(name="psd", bufs=1, space="PSUM"))

        w1_sb = consts.tile([128, KC, H], F16, tag="w1")
        nc.sync.dma_start(out=w1_sb,
                          in_=w1_d.ap().rearrange("(k p) c -> p k c", p=128))
        b1_sb = consts.tile([128, KC], F32, tag="b1")
        nc.sync.dma_start(out=b1_sb, in_=b1_d.ap())
        w2_sb = consts.tile([128, KC * 2], F16, tag="w2")
        nc.sync.dma_start(out=w2_sb, in_=w2_d.ap())
        b2_sb = consts.tile([128, 1], F32, tag="b2")
        nc.sync.dma_start(out=b2_sb, in_=b2_d.ap())
        io_sb = consts.tile([128, 128], F16, tag="iota")
        nc.sync.dma_start(out=io_sb, in_=io_d.ap())
        bv_sb = consts.tile([128, T], F32, tag="bv")
        nc.sync.dma_start(out=bv_sb, in_=bv_d.ap())
        ones_sb = consts.tile([128, 2], F16, tag="ones")
        nc.vector.memset(ones_sb, 1.0)
        # gate/e vectors, split at tile T1 so early pooling only depends
        # on the first exp batch.
        glo = consts.tile([128, T1], F16, tag="glo")
        ghi = consts.tile([128, T - T1], F16, tag="ghi")
        elo = consts.tile([128, T1], F32, tag="elo")
        ehi = consts.tile([128, T - T1], F32, tag="ehi")

        loop_cm = tc.For_i(0, reps, 1) if reps > 1 else nullcontext()
        with loop_cm:
            a1_tiles = {}
            pp = psp.tile([128, H], F32, tag="pp")
            pd = psd.tile([128, 2], F32, tag="pd")
            n_pool_done = 0

            def do_gate_block(su):
                """pg matmuls for supertile su -> gate columns [4su,4su+4)."""
                a1b = a1_tiles.pop(su)
                pg = psg.tile([128, 2 * KC], F32, tag="pg")
                for nch in range(4):
                    for d in range(KC):
                        nc.tensor.matmul(
                            out=pg[:, 2 * nch:2 * nch + 2],
                            lhsT=a1b[:, d, nch * 128:(nch + 1) * 128],
                            rhs=w2_sb[:, 2 * d:2 * d + 2],
                            start=(d == 0), stop=(d == KC - 1))
                t0 = 4 * su
                if t0 < T1:
                    nc.vector.tensor_copy(out=glo[:, t0:t0 + 4],
                                          in_=pg[:, 0:2 * KC:2])
                else:
                    nc.vector.tensor_copy(out=ghi[:, t0 - T1:t0 - T1 + 4],
                                          in_=pg[:, 0:2 * KC:2])

            def do_pool(pu, hp_tiles):
                """ms build + pp/pd matmuls for the 4 tiles of supertile pu."""
                nonlocal n_pool_done
                hpb = hp_tiles.pop(pu)
                for j in range(4):
                    t = 4 * pu + j
                    e_ap = (elo[:, t:t + 1] if t < T1
                            else ehi[:, t - T1:t - T1 + 1])
                    ms = ms_pool.tile([128, 128], F16, tag="ms")
                    nc.vector.tensor_scalar(
                        out=ms, in0=io_sb,
                        scalar1=bv_sb[:, t:t + 1], scalar2=e_ap,
                        op0=mybir.AluOpType.is_equal,
                        op1=mybir.AluOpType.mult)
                    nc.tensor.matmul(out=pp, lhsT=ms, rhs=hpb[:, j, :],
                                     start=(n_pool_done == 0),
                                     stop=(n_pool_done == T - 1))
                    nc.tensor.matmul(out=pd, lhsT=ms, rhs=ones_sb,
                                     start=(n_pool_done == 0),
                                     stop=(n_pool_done == T - 1))
                    n_pool_done += 1

            ht_tiles = {}
            hp_tiles = {}

            def issue_ht(s):
                if s < S and ablate != "noA":
                    htb = ht_pool.tile([128, KC, 512], F8E3, tag="ht")
                    nc.sync.dma_start(
                        out=htb,
                        in_=ht_d.ap()[s].rearrange("p (k n) -> p k n", k=KC))
                    ht_tiles[s] = htb

            def issue_hp(s):
                if s < S and ablate != "noC":
                    hpb = hp_pool.tile([128, 4, H], F16, tag="hp")
                    nc.sync.dma_start(
                        out=hpb,
                        in_=hp_d.ap().rearrange(
                            "(s j p) d -> s p j d", p=128, j=4)[s])
                    hp_tiles[s] = hpb

            for s in range(min(HT_BUFS - 1, S)):
                issue_ht(s)

            for s in range(S):
                issue_ht(s + HT_BUFS - 1)
                issue_hp(s)
                if ablate in ("noA", "dmaonly"):
                    pass
                else:
                    htb = ht_tiles.pop(s)
                    a1b = a1_pool.tile([128, KC, 512], F16, tag="a1")
                    if no_bias:
                        # b1 == 0: gelu in 2-chunk batches (one 2-bank PSUM
                        # tile per pair) to amortize the ~352-cycle ACT ramp
                        for i in range(2):
                            pz2 = psz.tile([128, 2, 512], F32, tag="pz")
                            for d2 in range(2):
                                d = 2 * i + d2
                                for k in range(KC):
                                    nc.tensor.matmul(
                                        out=pz2[:, d2, :],
                                        lhsT=w1_sb[:, k,
                                                   d * 128:(d + 1) * 128],
                                        rhs=htb[:, k, :],
                                        start=(k == 0), stop=(k == KC - 1))
                            if ablate == "noact":
                                nc.vector.tensor_copy(
                                    out=a1b[:, 2 * i:2 * i + 2, :], in_=pz2)
                            else:
                                nc.scalar.activation(
                                    out=a1b[:, 2 * i:2 * i + 2, :], in_=pz2,
                                    func=gelu_func, scale=1.0)
                    else:
                        for d in range(KC):
                            pz = psz.tile([128, 512], F32, tag="pz")
                            for k in range(KC):
                                nc.tensor.matmul(
                                    out=pz,
                                    lhsT=w1_sb[:, k, d * 128:(d + 1) * 128],
                                    rhs=htb[:, k, :],
                                    start=(k == 0), stop=(k == KC - 1))
                            if ablate == "noact":
                                nc.vector.tensor_copy(out=a1b[:, d, :],
                                                      in_=pz)
                            else:
                                nc.scalar.activation(out=a1b[:, d, :],
                                                     in_=pz,
                                                     func=gelu_func,
                                                     bias=b1_sb[:, d:d + 1],
                                                     scale=1.0)
                    a1_tiles[s] = a1b
                    if s >= 1 and ablate != "nogate":
                        do_gate_block(s - 1)
                    if s == SPLIT and ablate in ("", "noact"):
                        nc.scalar.activation(out=elo, in_=glo, func=exp_func,
                                             bias=b2_sb[:, 0:1], scale=1.0)
                if ablate in ("", "noact"):
                    for pu in pool_sched[s]:
                        do_pool(pu, hp_tiles)

            # ---- tail ----
            if ablate in ("", "noact"):
                do_gate_block(S - 1)
                nc.scalar.activation(out=ehi, in_=ghi, func=exp_func,
                                     bias=b2_sb[:, 0:1], scale=1.0)
                for pu in tail_pus:
                    do_pool(pu, hp_tiles)
            elif ablate == "noC":
                do_gate_block(S - 1)
            elif ablate in ("noA", "nogate"):
                # timing-only ablations: fake gates, pool everything here
                nc.vector.memset(glo, 0.125)
                nc.vector.memset(ghi, 0.125)
                nc.scalar.activation(out=elo, in_=glo, func=exp_func,
                                     bias=b2_sb[:, 0:1], scale=1.0)
                nc.scalar.activation(out=ehi, in_=ghi, func=exp_func,
                                     bias=b2_sb[:, 0:1], scale=1.0)
                for pu in sorted(hp_tiles):
                    do_pool(pu, hp_tiles)

            osb = small.tile([128, H], F32, tag="osb")
            if ablate in ("noC", "dmaonly"):
                nc.vector.memset(osb, 0.0)
            else:
                dcl = small.tile([128, 1], F32, tag="dcl")
                nc.vector.tensor_scalar(out=dcl, in0=pd[:, 0:1], scalar1=1e-35,
                                        scalar2=None, op0=mybir.AluOpType.max)
                rec = small.tile([128, 1], F32, tag="rec")
                nc.vector.reciprocal(out=rec, in_=dcl)
                nc.vector.tensor_scalar(out=osb, in0=pp, scalar1=rec[:, 0:1],
                                        scalar2=None,
                                        op0=mybir.AluOpType.mult)
            nc.sync.dma_start(out=out_d.ap(), in_=osb)

    nc.compile()
    return nc


_prog_cache: dict = {}


def _get_prog(np_pad: int, no_bias: bool):
    key = (np_pad, no_bias)
    if key not in _prog_cache:
        _prog_cache[key] = _build(np_pad, no_bias=no_bias)
    return _prog_cache[key]


def _prep_in_maps(h, bv, W1, b1, W2, b2, np_pad):
    """Shard + pad inputs per core; returns list of per-core input dicts."""
    T = np_pad // 128
    S = np_pad // 512
    bounds = np.searchsorted(bv, np.arange(0, NUM_GRAPHS + 1, G))

    w1v = np.ascontiguousarray(W1.astype(np.float16))
    b1v = np.ascontiguousarray(b1.astype(np.float32).reshape(4, 128).T)
    w2v = np.zeros((128, KC * 2), np.float16)
    w2v[:, 0::2] = W2[:, 0].astype(np.float16).reshape(4, 128).T
    b2t = np.full((128, 1), np.float32(b2.reshape(-1)[0]), np.float32)
    iota = np.ascontiguousarray(
        np.tile(np.arange(128, dtype=np.float16), (128, 1)))

    in_maps = []
    for c in range(N_CORES):
        n0, n1 = int(bounds[c]), int(bounds[c + 1])
        cnt = n1 - n0
        hp = np.zeros((np_pad, H), np.float16)
        hp[:cnt] = h[n0:n1].astype(np.float16)
        ht = np.zeros((H, np_pad), np.float32)
        ht[:, :cnt] = h[n0:n1].T
        # tiled hT: htq[s, p, k*512+c] = hT[k*128+p, s*512+c]  (2KB runs)
        htq = np.ascontiguousarray(
            ht.reshape(KC, 128, S, 512).transpose(2, 1, 0, 3)
            .reshape(S, 128, KC * 512).astype(NP_E3))
        bvrel = np.full(np_pad, -1.0, np.float32)
        bvrel[:cnt] = bv[n0:n1].astype(np.float32) - c * G
        bvrel = np.ascontiguousarray(bvrel.reshape(T, 128).T)
        in_maps.append({
            "htq": htq,
            "hp": np.ascontiguousarray(hp),
            "W1v": w1v,
            "b1v": b1v,
            "W2v": w2v,
            "b2t": b2t,
            "bvrel": bvrel,
            "iota": iota,
        })
    return in_maps


def kernel(**inputs) -> np.ndarray:
    h = np.ascontiguousarray(np.asarray(inputs["h"], dtype=np.float32))
    bv = np.asarray(inputs["batch_vec"]).astype(np.int64)
    W1 = np.asarray(inputs["W1"], dtype=np.float32)
    b1 = np.asarray(inputs["b1"], dtype=np.float32)
    W2 = np.asarray(inputs["W2"], dtype=np.float32)
    b2 = np.asarray(inputs["b2"], dtype=np.float32)

    bounds = np.searchsorted(bv, np.arange(0, NUM_GRAPHS + 1, G))
    max_cnt = int(np.diff(bounds).max())
    np_pad = NP_DEFAULT
    if max_cnt > np_pad:  # fallback for unexpected distributions
        np_pad = ((max_cnt + 511) // 512) * 512

    no_bias = not np.any(b1)
    nc = _get_prog(np_pad, no_bias)
    in_maps = _prep_in_maps(h, bv, W1, b1, W2, b2, np_pad)
    trace = bool(int(os.environ.get("AP_TRACE", "0")))
    res = run_bass_kernel_spmd(nc, in_maps, list(range(N_CORES)), trace=trace)
    global last_results
    last_results = res
    out = np.concatenate([res.results[c]["out"] for c in range(N_CORES)],
                         axis=0).astype(np.float32)
    return out


last_results = None
